# revision 43
# baseline (speedup 1.0000x reference)
"""Trainium2 Bass kernel for nn_BallQLoss: PointNet++-style ball query +
grouping + L1 mask loss, sharded over 8 NeuronCores.

Per core: one (batch, row-half) shard -> 2048 query rows x 4096 candidate
columns. Pipeline per 128-row block:
  PE:   P4[n,j] = 2*dot(pc_n,pc_j) - sq_j           (K=21 matmul, f32)
  ACT:  S = sign(P4 + (r^2 - sq_n))                 (+1 in-ball, -1 out)
  DVE:  keyed = S * nj  (nj = descending f16 ULP ladder; key encodes index)
        max8 -> top-8; keyed2 = (keyed < m8)*keyed; max8 -> ranks 9-16
        idx decode via f16-ULP bitcast; junk slots padded with slot-0 idx
  DMA:  wrap idx to ap_gather layout via DRAM round trip (4 quarters of 512
        (query,slot) pairs, each transposed + replicated to 2 Q7 cores)
  Pool: ONE ap_gather vs a 4x-replicated channel-transposed mask table
        [128, 4096] f32 in SBUF -> G4[128, 512] (partition = (rep, channel))
  DVE:  diff = G4 - own (broadcast over slots); abs-sum reduce -> acc col
Final: per-core scalar partial via ones-matmul partition reduce; host sums
partials and divides by (B*N*K).
"""
import os
import sys

import numpy as np

try:
    import concourse.bass as bass
except ImportError:
    sys.path.insert(0, '/opt/trn_rl_repo')
    import concourse.bass as bass

import concourse.mybir as mybir
import concourse.tile as tile
from concourse import bacc
from concourse.bass_utils import run_bass_kernel_spmd

f32 = mybir.dt.float32
f16 = mybir.dt.float16
bf16 = mybir.dt.bfloat16
u16 = mybir.dt.uint16
i16 = mybir.dt.int16
i32 = mybir.dt.int32
KDIM = 21  # 6 hi/mid/lo cross pairs x 3 dims + 3 split -sq rows
# f16 descending key table: nj[j] = bitcast_f16(NJ_BASE - j); consecutive f16
# ULPs are consecutive integer bit patterns, so j = NJ_BASE - bits(v).
NJ_BASE = 27648  # bits of f16(4096.0)

B = 4            # batches
N = 4096         # points per batch
C = 30           # mask channels
KN = 16          # neighbors per query
R2 = np.float32(0.2) * np.float32(0.2)
NCORES = 8
ROWS = 2048      # query rows per core (half a batch)
NBLK = ROWS // 128
NF = N // 512    # 512-wide column tiles per block

MULT_ON_POOL = os.environ.get("MULT_ON_POOL", "1") == "1"

_PROGRAM = None


def _build_program():
    nc = bacc.Bacc("TRN2", target_bir_lowering=False, debug=False)

    lhsT_d = nc.dram_tensor("lhsT", [KDIM, ROWS], bf16, kind="ExternalInput")
    rhs_d = nc.dram_tensor("rhs", [KDIM, N], bf16, kind="ExternalInput")
    nthr_d = nc.dram_tensor("nthr", [128, NBLK], f32, kind="ExternalInput")
    nj_d = nc.dram_tensor("nj", [N], f16, kind="ExternalInput")
    maskT4_d = nc.dram_tensor("maskT4", [128, N], f32, kind="ExternalInput")
    ownQ_d = nc.dram_tensor("ownQ", [128, NBLK * 32], f32,
                            kind="ExternalInput")
    partial_d = nc.dram_tensor("partial", [1, 1], f32, kind="ExternalOutput")

    with tile.TileContext(nc) as tc:
        with (
            tc.tile_pool(name="const", bufs=1) as const_pool,
            tc.tile_pool(name="psum", bufs=7, space="PSUM") as psum_pool,
            tc.tile_pool(name="psumf", bufs=1, space="PSUM") as psumf_pool,
            tc.tile_pool(name="sbS", bufs=4) as s_pool,
            tc.tile_pool(name="sbK", bufs=4) as k_pool,
            tc.tile_pool(name="sbK2", bufs=4) as k2_pool,
            tc.tile_pool(name="small", bufs=6) as small_pool,
            tc.tile_pool(name="idxp", bufs=7) as idx_pool,
            tc.tile_pool(name="gat", bufs=10) as gat_pool,
            tc.tile_pool(name="dif", bufs=3) as dif_pool,
        ):
            lhsT = const_pool.tile([KDIM, ROWS], bf16)
            nc.sync.dma_start(lhsT[:], lhsT_d[:])
            rhs = const_pool.tile([KDIM, N], bf16)
            nc.sync.dma_start(rhs[:], rhs_d[:])
            nthr = const_pool.tile([128, NBLK], f32)
            nc.sync.dma_start(nthr[:], nthr_d[:])
            nj = const_pool.tile([128, N], f16)
            nc.sync.dma_start(nj[:], bass.AP(nj_d, 0, [[0, 128], [1, N]]))
            maskT4 = const_pool.tile([128, N], f32)
            nc.sync.dma_start(maskT4[:], maskT4_d[:])
            ownQ = const_pool.tile([128, NBLK * 32], f32)
            nc.sync.dma_start(ownQ[:], ownQ_d[:])
            acc = const_pool.tile([128, NBLK], f32)

            keyed_tiles = {}

            def produce(blk):
                """PE matmuls + ACT sign + Pool multiply -> keyed[blk]."""
                S = s_pool.tile([128, N], f16)
                for f in range(NF):
                    fs = slice(f * 512, (f + 1) * 512)
                    p = psum_pool.tile([128, 512], f32)
                    nc.tensor.matmul(p[:], lhsT[:, blk * 128:(blk + 1) * 128],
                                     rhs[:, fs])
                    nc.scalar.activation(S[:, fs], p[:],
                                         mybir.ActivationFunctionType.Sign,
                                         bias=nthr[:, blk:blk + 1], scale=1.0)
                keyed = k_pool.tile([128, N], f16)
                if MULT_ON_POOL:
                    nc.gpsimd.tensor_tensor(out=keyed[:], in0=S[:],
                                            in1=nj[:],
                                            op=mybir.AluOpType.mult)
                else:
                    nc.vector.tensor_tensor(out=keyed[:], in0=S[:],
                                            in1=nj[:],
                                            op=mybir.AluOpType.mult)
                keyed_tiles[blk] = keyed

            pending_loss = []  # [(G4, blk)] deferred |diff| reduces
            LOSS_LAG = 5  # hide the Pool engine's ~10us wake-up latency

            def flush_loss(limit):
                while len(pending_loss) > limit:
                    G4, pblk = pending_loss.pop(0)
                    own_b = ownQ[:, pblk * 32:(pblk + 1) * 32].unsqueeze(2) \
                        .broadcast_to((128, 32, KN))
                    diff = dif_pool.tile([128, 32, KN], f32)
                    nc.vector.tensor_tensor(
                        out=diff[:],
                        in0=G4[:].rearrange("p (t s) -> p t s", t=32),
                        in1=own_b, op=mybir.AluOpType.subtract)
                    nc.vector.reduce_sum(acc[:, pblk:pblk + 1], diff[:],
                                         mybir.AxisListType.XY,
                                         apply_absolute_value=True)

            produce(0)
            produce(1)
            for blk in range(NBLK):
                keyed = keyed_tiles.pop(blk)

                v16 = small_pool.tile([128, KN], f16, tag="v16")
                nc.vector.max(v16[:, 0:8], keyed[:])
                # mask top-8: keyed2 = keyed - 60000*(keyed >= m8)
                m8f = small_pool.tile([128, 1], f32, tag="m8f")
                nc.vector.tensor_copy(m8f[:], v16[:, 7:8])
                tmask = k2_pool.tile([128, N], f16, tag="tmask")
                nc.vector.tensor_scalar(out=tmask[:], in0=keyed[:],
                                        scalar1=m8f[:], scalar2=-60000.0,
                                        op0=mybir.AluOpType.is_ge,
                                        op1=mybir.AluOpType.mult)
                keyed2 = k2_pool.tile([128, N], f16, tag="keyed2")
                nc.vector.tensor_tensor(out=keyed2[:], in0=keyed[:],
                                        in1=tmask[:],
                                        op=mybir.AluOpType.add)
                nc.vector.max(v16[:, 8:16], keyed2[:])

                # idx decode: idx = NJ_BASE - bits(v); junk (v<=0) -> slot-0
                bits = small_pool.tile([128, KN], f32, tag="bits")
                nc.vector.tensor_copy(bits[:], v16[:].bitcast(u16))
                idxr = small_pool.tile([128, KN], f32, tag="idxr")
                nc.vector.tensor_scalar(out=idxr[:], in0=bits[:],
                                        scalar1=-1.0, scalar2=float(NJ_BASE),
                                        op0=mybir.AluOpType.mult,
                                        op1=mybir.AluOpType.add)
                m = small_pool.tile([128, KN], f32, tag="m")
                nc.vector.tensor_scalar(out=m[:], in0=v16[:], scalar1=0.0,
                                        scalar2=None,
                                        op0=mybir.AluOpType.is_gt)
                dm = small_pool.tile([128, KN], f32, tag="dm")
                nc.vector.scalar_tensor_tensor(
                    out=dm[:], in0=idxr[:], scalar=idxr[:, 0:1], in1=m[:],
                    op0=mybir.AluOpType.subtract, op1=mybir.AluOpType.mult)
                # write decoded idx into both column halves, then a 32x32
                # block transpose directly yields ap_gather's wrapped layout:
                # idxs[32r + a, t] = idxi2[32r + t, a], a and a+16 identical.
                idxi2 = small_pool.tile([128, 2 * KN], i16, tag="idxi2")
                nc.vector.tensor_scalar(out=idxi2[:, 0:KN], in0=dm[:],
                                        scalar1=idxr[:, 0:1], scalar2=None,
                                        op0=mybir.AluOpType.add)
                nc.vector.tensor_scalar(out=idxi2[:, KN:2 * KN], in0=dm[:],
                                        scalar1=idxr[:, 0:1], scalar2=None,
                                        op0=mybir.AluOpType.add)
                idxs = idx_pool.tile([128, 32], i16)
                nc.vector.transpose(idxs[:], idxi2[:])

                # gather: G4[32*r + c, t*16 + s] = maskT4[c, idx]
                G4 = gat_pool.tile([128, 512], f32)
                nc.gpsimd.ap_gather(
                    out_ap=G4[:].unsqueeze(2), in_ap=maskT4[:].unsqueeze(2),
                    idxs_ap=idxs[:], channels=128, num_elems=N, d=1,
                    num_idxs=512)

                pending_loss.append((G4, blk))
                flush_loss(min(LOSS_LAG, NBLK - 1 - blk))
                if blk + 2 < NBLK:
                    produce(blk + 2)
            flush_loss(0)

            rowtot = const_pool.tile([128, 1], f32)
            nc.vector.reduce_sum(rowtot[:], acc[:], mybir.AxisListType.X)
            ones = const_pool.tile([128, 1], f32)
            nc.vector.memset(ones[:], 1.0)
            ptot = psumf_pool.tile([1, 1], f32)
            nc.tensor.matmul(ptot[:], rowtot[:], ones[:])
            tot = const_pool.tile([1, 1], f32)
            nc.vector.tensor_copy(tot[:], ptot[:])
            nc.sync.dma_start(partial_d[:], tot[:])

    nc.compile()
    return nc


def _get_program():
    global _PROGRAM
    if _PROGRAM is None:
        _PROGRAM = _build_program()
    return _PROGRAM


try:
    import ml_dtypes
    _BF = ml_dtypes.bfloat16
except ImportError:
    _BF = None


def _split3(v):
    """f32 -> (hi, mid, lo) bf16 triplet with hi+mid+lo ~ v to ~2^-25 rel."""
    v = np.asarray(v, np.float32)
    h = v.astype(_BF)
    r = v - h.astype(np.float32)
    m = r.astype(_BF)
    l = (r - m.astype(np.float32)).astype(_BF)
    return h, m, l


def _make_in_maps(pc: np.ndarray, mask: np.ndarray):
    pc = np.asarray(pc, np.float32)
    mask = np.asarray(mask, np.float32)
    nj = (NJ_BASE - np.arange(N)).astype(np.uint16).view(np.float16)
    in_maps = []
    for core in range(NCORES):
        b, h = divmod(core, 2)
        rows = slice(h * ROWS, (h + 1) * ROWS)
        pcb = pc[b]                       # (N, 3)
        sq = np.sum(pcb * pcb, axis=1)    # (N,)
        # 3-way bf16 split of 2*pc_n (rows) and pc_j (cols); P4 accumulates
        # the 6 dominant cross products + split -sq_j rows in f32 PSUM.
        xh, xm, xl = _split3(2.0 * pcb[rows])
        yh, ym, yl = _split3(pcb)
        sh, sm, sl = _split3(sq)
        ones = np.ones((ROWS,), _BF)
        lhsT = np.stack([r for a, _ in ((xh, yh), (xh, ym), (xm, yh),
                                        (xh, yl), (xl, yh), (xm, ym))
                         for r in (a[:, 0], a[:, 1], a[:, 2])]
                        + [ones, ones, ones], axis=0)
        rhs = np.stack([r for _, bb in ((xh, yh), (xh, ym), (xm, yh),
                                        (xh, yl), (xl, yh), (xm, ym))
                        for r in (bb[:, 0], bb[:, 1], bb[:, 2])]
                       + [-sh, -sm, -sl], axis=0)
        nthr = (R2 - sq[rows]).reshape(NBLK, 128).T.copy()
        # 4x-replicated channel-transposed mask table [128, N]
        maskT4 = np.zeros((128, N), np.float32)
        for rep in range(4):
            maskT4[rep * 32:rep * 32 + C] = mask[b].T
        # quarter-aligned own view: ownQ[rep*32+c, blk*32+j] =
        # own[blk*128 + rep*32 + j, c]
        own = mask[b][rows]                            # (ROWS, C)
        oq = np.zeros((4, 32, NBLK, 32), np.float32)
        oq[:, :C] = own.reshape(NBLK, 4, 32, C).transpose(1, 3, 0, 2)
        ownQ = oq.reshape(128, NBLK * 32)
        in_maps.append({"lhsT": np.ascontiguousarray(lhsT),
                        "rhs": np.ascontiguousarray(rhs),
                        "nthr": np.ascontiguousarray(nthr),
                        "nj": nj,
                        "maskT4": maskT4,
                        "ownQ": np.ascontiguousarray(ownQ)})
    return in_maps


def _run(pc, mask, trace=False):
    nc = _get_program()
    in_maps = _make_in_maps(pc, mask)
    res = run_bass_kernel_spmd(nc, in_maps, list(range(NCORES)), trace=trace)
    total = sum(float(r["partial"][0, 0]) for r in res.results)
    loss = np.float32(total / (B * N * KN))
    return np.asarray(loss, dtype=np.float32), res


def kernel(pc, mask):
    loss, _ = _run(pc, mask)
    return loss


# revision 44
# speedup vs baseline: 1.0215x; 1.0215x over previous
"""Trainium2 Bass kernel for nn_BallQLoss: PointNet++-style ball query +
grouping + L1 mask loss, sharded over 8 NeuronCores.

Per core: one (batch, row-half) shard -> 2048 query rows x 4096 candidate
columns. Pipeline per 128-row block:
  PE:   P4[n,j] = 2*dot(pc_n,pc_j) - sq_j           (K=21 matmul, f32)
  ACT:  S = sign(P4 + (r^2 - sq_n))                 (+1 in-ball, -1 out)
  DVE:  keyed = S * nj  (nj = descending f16 ULP ladder; key encodes index)
        max8 -> top-8; keyed2 = (keyed < m8)*keyed; max8 -> ranks 9-16
        idx decode via f16-ULP bitcast; junk slots padded with slot-0 idx
  DMA:  wrap idx to ap_gather layout via DRAM round trip (4 quarters of 512
        (query,slot) pairs, each transposed + replicated to 2 Q7 cores)
  Pool: ONE ap_gather vs a 4x-replicated channel-transposed mask table
        [128, 4096] f32 in SBUF -> G4[128, 512] (partition = (rep, channel))
  DVE:  diff = G4 - own (broadcast over slots); abs-sum reduce -> acc col
Final: per-core scalar partial via ones-matmul partition reduce; host sums
partials and divides by (B*N*K).
"""
import os
import sys

import numpy as np

try:
    import concourse.bass as bass
except ImportError:
    sys.path.insert(0, '/opt/trn_rl_repo')
    import concourse.bass as bass

import concourse.mybir as mybir
import concourse.tile as tile
from concourse import bacc
from concourse.bass_utils import run_bass_kernel_spmd

f32 = mybir.dt.float32
f16 = mybir.dt.float16
bf16 = mybir.dt.bfloat16
u16 = mybir.dt.uint16
i16 = mybir.dt.int16
i32 = mybir.dt.int32
KDIM = 21  # 6 hi/mid/lo cross pairs x 3 dims + 3 split -sq rows
# f16 descending key table: nj[j] = bitcast_f16(NJ_BASE - j); consecutive f16
# ULPs are consecutive integer bit patterns, so j = NJ_BASE - bits(v).
NJ_BASE = 27648  # bits of f16(4096.0)

B = 4            # batches
N = 4096         # points per batch
C = 30           # mask channels
KN = 16          # neighbors per query
R2 = np.float32(0.2) * np.float32(0.2)
NCORES = 8
ROWS = 2048      # query rows per core (half a batch)
NBLK = ROWS // 128
NF = N // 512    # 512-wide column tiles per block

MULT_ON_POOL = os.environ.get("MULT_ON_POOL", "1") == "1"

_PROGRAM = None


def _build_program():
    nc = bacc.Bacc("TRN2", target_bir_lowering=False, debug=False)

    lhsT_d = nc.dram_tensor("lhsT", [KDIM, ROWS], bf16, kind="ExternalInput")
    rhs_d = nc.dram_tensor("rhs", [KDIM, N], bf16, kind="ExternalInput")
    nthr_d = nc.dram_tensor("nthr", [128, NBLK], f32, kind="ExternalInput")
    nj_d = nc.dram_tensor("nj", [N], f16, kind="ExternalInput")
    maskT4_d = nc.dram_tensor("maskT4", [128, N], f32, kind="ExternalInput")
    ownQ_d = nc.dram_tensor("ownQ", [128, NBLK * 32], f32,
                            kind="ExternalInput")
    partial_d = nc.dram_tensor("partial", [1, 1], f32, kind="ExternalOutput")

    with tile.TileContext(nc) as tc:
        with (
            tc.tile_pool(name="const", bufs=1) as const_pool,
            tc.tile_pool(name="psum", bufs=7, space="PSUM") as psum_pool,
            tc.tile_pool(name="psumf", bufs=1, space="PSUM") as psumf_pool,
            tc.tile_pool(name="sbS", bufs=4) as s_pool,
            tc.tile_pool(name="sbK", bufs=4) as k_pool,
            tc.tile_pool(name="sbK2", bufs=4) as k2_pool,
            tc.tile_pool(name="small", bufs=6) as small_pool,
            tc.tile_pool(name="idxp", bufs=7) as idx_pool,
            tc.tile_pool(name="gat", bufs=10) as gat_pool,
            tc.tile_pool(name="dif", bufs=3) as dif_pool,
        ):
            lhsT = const_pool.tile([KDIM, ROWS], bf16)
            nc.sync.dma_start(lhsT[:], lhsT_d[:])
            rhs = const_pool.tile([KDIM, N], bf16)
            nc.sync.dma_start(rhs[:], rhs_d[:])
            nthr = const_pool.tile([128, NBLK], f32)
            nc.sync.dma_start(nthr[:], nthr_d[:])
            nj = const_pool.tile([128, N], f16)
            nc.sync.dma_start(nj[:], bass.AP(nj_d, 0, [[0, 128], [1, N]]))
            maskT4 = const_pool.tile([128, N], f32)
            nc.sync.dma_start(maskT4[:], maskT4_d[:])
            ownQ = const_pool.tile([128, NBLK * 32], f32)
            nc.sync.dma_start(ownQ[:], ownQ_d[:])
            acc = const_pool.tile([128, NBLK], f32)

            keyed_tiles = {}

            def produce(blk):
                """PE matmuls + ACT sign + Pool multiply -> keyed[blk]."""
                S = s_pool.tile([128, N], f16)
                for f in range(NF):
                    fs = slice(f * 512, (f + 1) * 512)
                    p = psum_pool.tile([128, 512], f32)
                    nc.tensor.matmul(p[:], lhsT[:, blk * 128:(blk + 1) * 128],
                                     rhs[:, fs])
                    nc.scalar.activation(S[:, fs], p[:],
                                         mybir.ActivationFunctionType.Sign,
                                         bias=nthr[:, blk:blk + 1], scale=1.0)
                keyed = k_pool.tile([128, N], f16)
                if MULT_ON_POOL:
                    nc.gpsimd.tensor_tensor(out=keyed[:], in0=S[:],
                                            in1=nj[:],
                                            op=mybir.AluOpType.mult)
                else:
                    nc.vector.tensor_tensor(out=keyed[:], in0=S[:],
                                            in1=nj[:],
                                            op=mybir.AluOpType.mult)
                keyed_tiles[blk] = keyed

            pending_loss = []  # [(G4, blk)] deferred |diff| reduces
            LOSS_LAG = 5  # hide the Pool engine's ~10us wake-up latency

            def flush_loss(limit):
                while len(pending_loss) > limit:
                    G4, pblk = pending_loss.pop(0)
                    own_b = ownQ[:, pblk * 32:(pblk + 1) * 32].unsqueeze(2) \
                        .broadcast_to((128, 32, KN))
                    diff = dif_pool.tile([128, 32, KN], f32)
                    nc.vector.tensor_tensor(
                        out=diff[:],
                        in0=G4[:].rearrange("p (t s) -> p t s", t=32),
                        in1=own_b, op=mybir.AluOpType.subtract)
                    nc.vector.reduce_sum(acc[:, pblk:pblk + 1], diff[:],
                                         mybir.AxisListType.XY,
                                         apply_absolute_value=True)

            produce(0)
            produce(1)
            for blk in range(NBLK):
                if blk + 2 < NBLK:
                    produce(blk + 2)
                keyed = keyed_tiles.pop(blk)

                v16 = small_pool.tile([128, KN], f16, tag="v16")
                nc.vector.max(v16[:, 0:8], keyed[:])
                # mask top-8: keyed2 = keyed - 60000*(keyed >= m8)
                m8f = small_pool.tile([128, 1], f32, tag="m8f")
                nc.vector.tensor_copy(m8f[:], v16[:, 7:8])
                tmask = k2_pool.tile([128, N], f16, tag="tmask")
                nc.vector.tensor_scalar(out=tmask[:], in0=keyed[:],
                                        scalar1=m8f[:], scalar2=-60000.0,
                                        op0=mybir.AluOpType.is_ge,
                                        op1=mybir.AluOpType.mult)
                keyed2 = k2_pool.tile([128, N], f16, tag="keyed2")
                nc.vector.tensor_tensor(out=keyed2[:], in0=keyed[:],
                                        in1=tmask[:],
                                        op=mybir.AluOpType.add)
                nc.vector.max(v16[:, 8:16], keyed2[:])

                # idx decode: idx = NJ_BASE - bits(v); junk (v<=0) -> slot-0
                bits = small_pool.tile([128, KN], f32, tag="bits")
                nc.vector.tensor_copy(bits[:], v16[:].bitcast(u16))
                idxr = small_pool.tile([128, KN], f32, tag="idxr")
                nc.vector.tensor_scalar(out=idxr[:], in0=bits[:],
                                        scalar1=-1.0, scalar2=float(NJ_BASE),
                                        op0=mybir.AluOpType.mult,
                                        op1=mybir.AluOpType.add)
                m = small_pool.tile([128, KN], f32, tag="m")
                nc.vector.tensor_scalar(out=m[:], in0=v16[:], scalar1=0.0,
                                        scalar2=None,
                                        op0=mybir.AluOpType.is_gt)
                dm = small_pool.tile([128, KN], f32, tag="dm")
                nc.vector.scalar_tensor_tensor(
                    out=dm[:], in0=idxr[:], scalar=idxr[:, 0:1], in1=m[:],
                    op0=mybir.AluOpType.subtract, op1=mybir.AluOpType.mult)
                # write decoded idx into both column halves, then a 32x32
                # block transpose directly yields ap_gather's wrapped layout:
                # idxs[32r + a, t] = idxi2[32r + t, a], a and a+16 identical.
                idxi2 = small_pool.tile([128, 2 * KN], i16, tag="idxi2")
                nc.vector.tensor_scalar(out=idxi2[:, 0:KN], in0=dm[:],
                                        scalar1=idxr[:, 0:1], scalar2=None,
                                        op0=mybir.AluOpType.add)
                nc.vector.tensor_scalar(out=idxi2[:, KN:2 * KN], in0=dm[:],
                                        scalar1=idxr[:, 0:1], scalar2=None,
                                        op0=mybir.AluOpType.add)
                idxs = idx_pool.tile([128, 32], i16)
                nc.vector.transpose(idxs[:], idxi2[:])

                # gather: G4[32*r + c, t*16 + s] = maskT4[c, idx]
                G4 = gat_pool.tile([128, 512], f32)
                nc.gpsimd.ap_gather(
                    out_ap=G4[:].unsqueeze(2), in_ap=maskT4[:].unsqueeze(2),
                    idxs_ap=idxs[:], channels=128, num_elems=N, d=1,
                    num_idxs=512)

                pending_loss.append((G4, blk))
                flush_loss(min(LOSS_LAG, NBLK - 1 - blk))
            flush_loss(0)

            rowtot = const_pool.tile([128, 1], f32)
            nc.vector.reduce_sum(rowtot[:], acc[:], mybir.AxisListType.X)
            ones = const_pool.tile([128, 1], f32)
            nc.vector.memset(ones[:], 1.0)
            ptot = psumf_pool.tile([1, 1], f32)
            nc.tensor.matmul(ptot[:], rowtot[:], ones[:])
            tot = const_pool.tile([1, 1], f32)
            nc.vector.tensor_copy(tot[:], ptot[:])
            nc.sync.dma_start(partial_d[:], tot[:])

    nc.compile()
    return nc


def _get_program():
    global _PROGRAM
    if _PROGRAM is None:
        _PROGRAM = _build_program()
    return _PROGRAM


try:
    import ml_dtypes
    _BF = ml_dtypes.bfloat16
except ImportError:
    _BF = None


def _split3(v):
    """f32 -> (hi, mid, lo) bf16 triplet with hi+mid+lo ~ v to ~2^-25 rel."""
    v = np.asarray(v, np.float32)
    h = v.astype(_BF)
    r = v - h.astype(np.float32)
    m = r.astype(_BF)
    l = (r - m.astype(np.float32)).astype(_BF)
    return h, m, l


def _make_in_maps(pc: np.ndarray, mask: np.ndarray):
    pc = np.asarray(pc, np.float32)
    mask = np.asarray(mask, np.float32)
    nj = (NJ_BASE - np.arange(N)).astype(np.uint16).view(np.float16)
    in_maps = []
    for core in range(NCORES):
        b, h = divmod(core, 2)
        rows = slice(h * ROWS, (h + 1) * ROWS)
        pcb = pc[b]                       # (N, 3)
        sq = np.sum(pcb * pcb, axis=1)    # (N,)
        # 3-way bf16 split of 2*pc_n (rows) and pc_j (cols); P4 accumulates
        # the 6 dominant cross products + split -sq_j rows in f32 PSUM.
        xh, xm, xl = _split3(2.0 * pcb[rows])
        yh, ym, yl = _split3(pcb)
        sh, sm, sl = _split3(sq)
        ones = np.ones((ROWS,), _BF)
        lhsT = np.stack([r for a, _ in ((xh, yh), (xh, ym), (xm, yh),
                                        (xh, yl), (xl, yh), (xm, ym))
                         for r in (a[:, 0], a[:, 1], a[:, 2])]
                        + [ones, ones, ones], axis=0)
        rhs = np.stack([r for _, bb in ((xh, yh), (xh, ym), (xm, yh),
                                        (xh, yl), (xl, yh), (xm, ym))
                        for r in (bb[:, 0], bb[:, 1], bb[:, 2])]
                       + [-sh, -sm, -sl], axis=0)
        nthr = (R2 - sq[rows]).reshape(NBLK, 128).T.copy()
        # 4x-replicated channel-transposed mask table [128, N]
        maskT4 = np.zeros((128, N), np.float32)
        for rep in range(4):
            maskT4[rep * 32:rep * 32 + C] = mask[b].T
        # quarter-aligned own view: ownQ[rep*32+c, blk*32+j] =
        # own[blk*128 + rep*32 + j, c]
        own = mask[b][rows]                            # (ROWS, C)
        oq = np.zeros((4, 32, NBLK, 32), np.float32)
        oq[:, :C] = own.reshape(NBLK, 4, 32, C).transpose(1, 3, 0, 2)
        ownQ = oq.reshape(128, NBLK * 32)
        in_maps.append({"lhsT": np.ascontiguousarray(lhsT),
                        "rhs": np.ascontiguousarray(rhs),
                        "nthr": np.ascontiguousarray(nthr),
                        "nj": nj,
                        "maskT4": maskT4,
                        "ownQ": np.ascontiguousarray(ownQ)})
    return in_maps


def _run(pc, mask, trace=False):
    nc = _get_program()
    in_maps = _make_in_maps(pc, mask)
    res = run_bass_kernel_spmd(nc, in_maps, list(range(NCORES)), trace=trace)
    total = sum(float(r["partial"][0, 0]) for r in res.results)
    loss = np.float32(total / (B * N * KN))
    return np.asarray(loss, dtype=np.float32), res


def kernel(pc, mask):
    loss, _ = _run(pc, mask)
    return loss


# revision 45
# speedup vs baseline: 1.1789x; 1.1541x over previous
"""Trainium2 Bass kernel for nn_BallQLoss: PointNet++-style ball query +
grouping + L1 mask loss, sharded over 8 NeuronCores.

Per core: one (batch, row-half) shard -> 2048 query rows x 4096 candidate
columns. Pipeline per 128-row block:
  PE:   P4[n,j] = 2*dot(pc_n,pc_j) - sq_j           (K=21 matmul, f32)
  ACT:  S = sign(P4 + (r^2 - sq_n))                 (+1 in-ball, -1 out)
  DVE:  keyed = S * nj  (nj = descending f16 ULP ladder; key encodes index)
        max8 -> top-8; keyed2 = (keyed < m8)*keyed; max8 -> ranks 9-16
        idx decode via f16-ULP bitcast; junk slots padded with slot-0 idx
  DMA:  wrap idx to ap_gather layout via DRAM round trip (4 quarters of 512
        (query,slot) pairs, each transposed + replicated to 2 Q7 cores)
  Pool: ONE ap_gather vs a 4x-replicated channel-transposed mask table
        [128, 4096] f32 in SBUF -> G4[128, 512] (partition = (rep, channel))
  DVE:  diff = G4 - own (broadcast over slots); abs-sum reduce -> acc col
Final: per-core scalar partial via ones-matmul partition reduce; host sums
partials and divides by (B*N*K).
"""
import os
import sys

import numpy as np

try:
    import concourse.bass as bass
except ImportError:
    sys.path.insert(0, '/opt/trn_rl_repo')
    import concourse.bass as bass

import concourse.mybir as mybir
import concourse.tile as tile
from concourse import bacc
from concourse.bass_utils import run_bass_kernel_spmd

f32 = mybir.dt.float32
f16 = mybir.dt.float16
bf16 = mybir.dt.bfloat16
u16 = mybir.dt.uint16
i16 = mybir.dt.int16
i32 = mybir.dt.int32
KDIM = 21  # 6 hi/mid/lo cross pairs x 3 dims + 3 split -sq rows
# f16 descending key table: nj[j] = bitcast_f16(NJ_BASE - j); consecutive f16
# ULPs are consecutive integer bit patterns, so j = NJ_BASE - bits(v).
NJ_BASE = 27648  # bits of f16(4096.0)

B = 4            # batches
N = 4096         # points per batch
C = 30           # mask channels
KN = 16          # neighbors per query
R2 = np.float32(0.2) * np.float32(0.2)
NCORES = 8
ROWS = 2048      # query rows per core (half a batch)
NBLK = ROWS // 128
NF = N // 512    # 512-wide column tiles per block

MULT_ON_POOL = os.environ.get("MULT_ON_POOL", "1") == "1"

_PROGRAM = None


def _build_program():
    nc = bacc.Bacc("TRN2", target_bir_lowering=False, debug=False)

    lhsT_d = nc.dram_tensor("lhsT", [KDIM, ROWS], bf16, kind="ExternalInput")
    rhs_d = nc.dram_tensor("rhs", [KDIM, N], bf16, kind="ExternalInput")
    nthr_d = nc.dram_tensor("nthr", [128, NBLK], f32, kind="ExternalInput")
    nj_d = nc.dram_tensor("nj", [N], f16, kind="ExternalInput")
    maskT4_d = nc.dram_tensor("maskT4", [128, N], f32, kind="ExternalInput")
    ownQ_d = nc.dram_tensor("ownQ", [128, NBLK * 32], f32,
                            kind="ExternalInput")
    partial_d = nc.dram_tensor("partial", [1, 1], f32, kind="ExternalOutput")

    with tile.TileContext(nc) as tc:
        with (
            tc.tile_pool(name="const", bufs=1) as const_pool,
            tc.tile_pool(name="psum", bufs=7, space="PSUM") as psum_pool,
            tc.tile_pool(name="psumf", bufs=1, space="PSUM") as psumf_pool,
            tc.tile_pool(name="sbS", bufs=4) as s_pool,
            tc.tile_pool(name="sbK", bufs=4) as k_pool,
            tc.tile_pool(name="sbK2", bufs=4) as k2_pool,
            tc.tile_pool(name="small", bufs=6) as small_pool,
            tc.tile_pool(name="idxp", bufs=7) as idx_pool,
            tc.tile_pool(name="gat", bufs=10) as gat_pool,
            tc.tile_pool(name="dif", bufs=3) as dif_pool,
        ):
            lhsT = const_pool.tile([KDIM, ROWS], bf16)
            nc.sync.dma_start(lhsT[:], lhsT_d[:])
            rhs = const_pool.tile([KDIM, N], bf16)
            nc.sync.dma_start(rhs[:], rhs_d[:])
            nthr = const_pool.tile([128, NBLK], f32)
            nc.sync.dma_start(nthr[:], nthr_d[:])
            nj = const_pool.tile([128, N], f16)
            nc.sync.dma_start(nj[:], bass.AP(nj_d, 0, [[0, 128], [1, N]]))
            maskT4 = const_pool.tile([128, N], f32)
            nc.sync.dma_start(maskT4[:], maskT4_d[:])
            ownQ = const_pool.tile([128, NBLK * 32], f32)
            nc.sync.dma_start(ownQ[:], ownQ_d[:])
            acc = const_pool.tile([128, NBLK], f32)

            keyed_tiles = {}

            def produce(blk):
                """PE matmuls + ACT sign + Pool multiply -> keyed[blk]."""
                S = s_pool.tile([128, N], f16)
                for f in range(NF):
                    fs = slice(f * 512, (f + 1) * 512)
                    p = psum_pool.tile([128, 512], f32)
                    nc.tensor.matmul(p[:], lhsT[:, blk * 128:(blk + 1) * 128],
                                     rhs[:, fs])
                    nc.scalar.activation(S[:, fs], p[:],
                                         mybir.ActivationFunctionType.Sign,
                                         bias=nthr[:, blk:blk + 1], scale=1.0)
                keyed = k_pool.tile([128, N], f16)
                if MULT_ON_POOL:
                    nc.gpsimd.tensor_tensor(out=keyed[:], in0=S[:],
                                            in1=nj[:],
                                            op=mybir.AluOpType.mult)
                else:
                    nc.vector.tensor_tensor(out=keyed[:], in0=S[:],
                                            in1=nj[:],
                                            op=mybir.AluOpType.mult)
                keyed_tiles[blk] = keyed

            pending_loss = []  # [(G4, blk)] deferred |diff| reduces
            LOSS_LAG = 5  # hide the Pool engine's ~10us wake-up latency

            def flush_loss(limit):
                while len(pending_loss) > limit:
                    G4, pblk = pending_loss.pop(0)
                    own_b = ownQ[:, pblk * 32:(pblk + 1) * 32].unsqueeze(2) \
                        .broadcast_to((128, 32, KN))
                    diff = dif_pool.tile([128, 32, KN], f32)
                    nc.vector.tensor_tensor(
                        out=diff[:],
                        in0=G4[:].rearrange("p (t s) -> p t s", t=32),
                        in1=own_b, op=mybir.AluOpType.subtract)
                    nc.vector.reduce_sum(acc[:, pblk:pblk + 1], diff[:],
                                         mybir.AxisListType.XY,
                                         apply_absolute_value=True)

            produce(0)
            produce(1)
            for blk in range(NBLK):
                if blk + 2 < NBLK:
                    produce(blk + 2)
                keyed = keyed_tiles.pop(blk)

                v16 = small_pool.tile([128, KN], f16, tag="v16")
                nc.vector.max(v16[:, 0:8], keyed[:])
                # mask top-8: keyed2 = keyed - 60000*(keyed >= m8)
                m8f = small_pool.tile([128, 1], f32, tag="m8f")
                nc.scalar.activation(m8f[:], v16[:, 7:8],
                                     mybir.ActivationFunctionType.Copy,
                                     bias=0.0, scale=1.0)
                tmask = k2_pool.tile([128, N], f16, tag="tmask")
                nc.vector.tensor_scalar(out=tmask[:], in0=keyed[:],
                                        scalar1=m8f[:], scalar2=-60000.0,
                                        op0=mybir.AluOpType.is_ge,
                                        op1=mybir.AluOpType.mult)
                keyed2 = k2_pool.tile([128, N], f16, tag="keyed2")
                nc.vector.tensor_tensor(out=keyed2[:], in0=keyed[:],
                                        in1=tmask[:],
                                        op=mybir.AluOpType.add)
                nc.vector.max(v16[:, 8:16], keyed2[:])

                # idx decode: idx = NJ_BASE - bits(v); junk (v<=0) -> slot-0
                bits = small_pool.tile([128, KN], f32, tag="bits")
                nc.vector.tensor_copy(bits[:], v16[:].bitcast(u16))
                idxr = small_pool.tile([128, KN], f32, tag="idxr")
                nc.vector.tensor_scalar(out=idxr[:], in0=bits[:],
                                        scalar1=-1.0, scalar2=float(NJ_BASE),
                                        op0=mybir.AluOpType.mult,
                                        op1=mybir.AluOpType.add)
                m = small_pool.tile([128, KN], f32, tag="m")
                nc.vector.tensor_scalar(out=m[:], in0=v16[:], scalar1=0.0,
                                        scalar2=None,
                                        op0=mybir.AluOpType.is_gt)
                dm = small_pool.tile([128, KN], f32, tag="dm")
                nc.vector.scalar_tensor_tensor(
                    out=dm[:], in0=idxr[:], scalar=idxr[:, 0:1], in1=m[:],
                    op0=mybir.AluOpType.subtract, op1=mybir.AluOpType.mult)
                # write decoded idx into both column halves, then a 32x32
                # block transpose directly yields ap_gather's wrapped layout:
                # idxs[32r + a, t] = idxi2[32r + t, a], a and a+16 identical.
                idxi2 = small_pool.tile([128, 2 * KN], i16, tag="idxi2")
                nc.vector.tensor_scalar(out=idxi2[:, 0:KN], in0=dm[:],
                                        scalar1=idxr[:, 0:1], scalar2=None,
                                        op0=mybir.AluOpType.add)
                nc.vector.tensor_scalar(out=idxi2[:, KN:2 * KN], in0=dm[:],
                                        scalar1=idxr[:, 0:1], scalar2=None,
                                        op0=mybir.AluOpType.add)
                idxs = idx_pool.tile([128, 32], i16)
                nc.vector.transpose(idxs[:], idxi2[:])

                # gather: G4[32*r + c, t*16 + s] = maskT4[c, idx]
                G4 = gat_pool.tile([128, 512], f32)
                nc.gpsimd.ap_gather(
                    out_ap=G4[:].unsqueeze(2), in_ap=maskT4[:].unsqueeze(2),
                    idxs_ap=idxs[:], channels=128, num_elems=N, d=1,
                    num_idxs=512)

                pending_loss.append((G4, blk))
                flush_loss(min(LOSS_LAG, NBLK - 1 - blk))
            flush_loss(0)

            rowtot = const_pool.tile([128, 1], f32)
            nc.vector.reduce_sum(rowtot[:], acc[:], mybir.AxisListType.X)
            ones = const_pool.tile([128, 1], f32)
            nc.vector.memset(ones[:], 1.0)
            ptot = psumf_pool.tile([1, 1], f32)
            nc.tensor.matmul(ptot[:], rowtot[:], ones[:])
            tot = const_pool.tile([1, 1], f32)
            nc.vector.tensor_copy(tot[:], ptot[:])
            nc.sync.dma_start(partial_d[:], tot[:])

    nc.compile()
    return nc


def _get_program():
    global _PROGRAM
    if _PROGRAM is None:
        _PROGRAM = _build_program()
    return _PROGRAM


try:
    import ml_dtypes
    _BF = ml_dtypes.bfloat16
except ImportError:
    _BF = None


def _split3(v):
    """f32 -> (hi, mid, lo) bf16 triplet with hi+mid+lo ~ v to ~2^-25 rel."""
    v = np.asarray(v, np.float32)
    h = v.astype(_BF)
    r = v - h.astype(np.float32)
    m = r.astype(_BF)
    l = (r - m.astype(np.float32)).astype(_BF)
    return h, m, l


def _make_in_maps(pc: np.ndarray, mask: np.ndarray):
    pc = np.asarray(pc, np.float32)
    mask = np.asarray(mask, np.float32)
    nj = (NJ_BASE - np.arange(N)).astype(np.uint16).view(np.float16)
    in_maps = []
    for core in range(NCORES):
        b, h = divmod(core, 2)
        rows = slice(h * ROWS, (h + 1) * ROWS)
        pcb = pc[b]                       # (N, 3)
        sq = np.sum(pcb * pcb, axis=1)    # (N,)
        # 3-way bf16 split of 2*pc_n (rows) and pc_j (cols); P4 accumulates
        # the 6 dominant cross products + split -sq_j rows in f32 PSUM.
        xh, xm, xl = _split3(2.0 * pcb[rows])
        yh, ym, yl = _split3(pcb)
        sh, sm, sl = _split3(sq)
        ones = np.ones((ROWS,), _BF)
        lhsT = np.stack([r for a, _ in ((xh, yh), (xh, ym), (xm, yh),
                                        (xh, yl), (xl, yh), (xm, ym))
                         for r in (a[:, 0], a[:, 1], a[:, 2])]
                        + [ones, ones, ones], axis=0)
        rhs = np.stack([r for _, bb in ((xh, yh), (xh, ym), (xm, yh),
                                        (xh, yl), (xl, yh), (xm, ym))
                        for r in (bb[:, 0], bb[:, 1], bb[:, 2])]
                       + [-sh, -sm, -sl], axis=0)
        nthr = (R2 - sq[rows]).reshape(NBLK, 128).T.copy()
        # 4x-replicated channel-transposed mask table [128, N]
        maskT4 = np.zeros((128, N), np.float32)
        for rep in range(4):
            maskT4[rep * 32:rep * 32 + C] = mask[b].T
        # quarter-aligned own view: ownQ[rep*32+c, blk*32+j] =
        # own[blk*128 + rep*32 + j, c]
        own = mask[b][rows]                            # (ROWS, C)
        oq = np.zeros((4, 32, NBLK, 32), np.float32)
        oq[:, :C] = own.reshape(NBLK, 4, 32, C).transpose(1, 3, 0, 2)
        ownQ = oq.reshape(128, NBLK * 32)
        in_maps.append({"lhsT": np.ascontiguousarray(lhsT),
                        "rhs": np.ascontiguousarray(rhs),
                        "nthr": np.ascontiguousarray(nthr),
                        "nj": nj,
                        "maskT4": maskT4,
                        "ownQ": np.ascontiguousarray(ownQ)})
    return in_maps


def _run(pc, mask, trace=False):
    nc = _get_program()
    in_maps = _make_in_maps(pc, mask)
    res = run_bass_kernel_spmd(nc, in_maps, list(range(NCORES)), trace=trace)
    total = sum(float(r["partial"][0, 0]) for r in res.results)
    loss = np.float32(total / (B * N * KN))
    return np.asarray(loss, dtype=np.float32), res


def kernel(pc, mask):
    loss, _ = _run(pc, mask)
    return loss
